# revision 14
# baseline (speedup 1.0000x reference)
"""Int32 3x3 conv2d (stride 1, pad 1) as bf16 matmuls on 8 TRN2 cores.

Problem: x[16,256,56,56] (*) w[256,256,3,3] + b[256] -> y[16,256,56,56],
all int32, values in [0,127).

Trick: values 0..126 are exactly representable in bf16, every product is
an integer < 2^14, and every accumulation stays < 2^24, so a bf16 matmul
with fp32 PSUM accumulation produces bit-exact integer results.

Layout: each image is zero-padded to 58x58. The 3x3 conv becomes 9
shifted [Cin,Cout]^T @ [Cin,pixels] matmuls accumulated in PSUM; pixel
tiles are 8 output rows x 56 cols = 448 columns (one PSUM bank), read
from the padded image through a strided access pattern so only valid
pixels are computed.

Sharding: data-parallel over batch, 2 images per core; weights replicated.
"""

import numpy as np
import ml_dtypes

B, C, H, W = 16, 256, 56, 56
HP, WP = H + 2, W + 2          # 58, 58 padded
IMG = HP * WP                  # 3364 flat padded image
N_CORES = 8
IMG_PER_CORE = B // N_CORES    # 2
ROWS_PER_CHUNK = 8
CHUNK = ROWS_PER_CHUNK * W     # 448 valid pixels, fits one PSUM bank
N_CHUNKS = H // ROWS_PER_CHUNK  # 7
N_WARM = 10                    # matmuls to flip the HAM clock gate and
                               # bridge the input-DMA window

_BF16 = ml_dtypes.bfloat16


def _build_program():
    import concourse.bass as bass
    import concourse.mybir as mybir
    from concourse import bacc
    from concourse.tile import TileContext

    nc = bacc.Bacc("TRN2", target_bir_lowering=False, debug=False)

    x_h = nc.dram_tensor(
        "x", [2, 128, IMG_PER_CORE * IMG], mybir.dt.bfloat16,
        kind="ExternalInput",
    )
    w_h = nc.dram_tensor(
        "w", [128, 2 * 2 * 9 * 128], mybir.dt.bfloat16, kind="ExternalInput"
    )
    b_h = nc.dram_tensor("b", [128, 2], mybir.dt.float32, kind="ExternalInput")
    y_h = nc.dram_tensor(
        "y", [IMG_PER_CORE, 2, 128, H, W], mybir.dt.int32, kind="ExternalOutput"
    )

    with TileContext(nc) as tc:
        with (
            tc.tile_pool(name="const", bufs=1) as const_pool,
            tc.tile_pool(name="xin", bufs=1) as x_pool,
            tc.tile_pool(name="psum", bufs=5, space="PSUM") as psum_pool,
            tc.tile_pool(name="warm", bufs=1, space="PSUM") as warm_pool,
            tc.tile_pool(name="outs", bufs=2) as out_pool,
        ):
            # PE warm-up: ~3.4us of junk matmuls on a zeroed tile while the
            # input DMAs land, so the HAM clock gate is at 8/8 (2.4 GHz)
            # when the real matmuls start.
            wz = const_pool.tile([128, 128 + CHUNK], mybir.dt.bfloat16)
            nc.vector.memset(wz[:, :], 0.0)
            wps = warm_pool.tile([128, CHUNK], mybir.dt.float32)
            for i in range(N_WARM):
                nc.tensor.matmul(
                    wps[:, :], wz[:, 0:128], wz[:, 128:128 + CHUNK],
                    start=True, stop=True,
                )

            # Input DMAs: one w tile per (ci_chunk, co_chunk) and one x tile
            # per (ci_chunk, img), so each matmul gates on exactly the data
            # it reads. Issues are spread across engine sequencers (a DMA
            # trigger costs ~0.6us of sequencer time) with the first matmul
            # group's tensors (w00, x00) issued first and in parallel.
            w_sb = {}
            for ci in range(2):
                for co in range(2):
                    w_sb[ci, co] = const_pool.tile(
                        [128, 9 * 128], mybir.dt.bfloat16,
                        tag=f"w_{ci}_{co}", name=f"w_{ci}_{co}",
                    )

            def w_dma(eng, ci, co):
                s = (ci * 2 + co) * 9 * 128
                eng.dma_start(w_sb[ci, co][:, :], w_h.ap()[:, s:s + 9 * 128])

            # x(0,0) is the gate for the very first matmul: split it into
            # two row-slabs so the PE only waits for rows 0..33 (the first
            # 4 output chunks) instead of the whole image.
            A_ROWS = 34          # rows 0..33, covers output rows 0..31
            B_ROWS = HP - 32     # rows 32..57, covers output rows 32..55
            x00a = x_pool.tile([128, A_ROWS * WP], mybir.dt.bfloat16)
            x00b = x_pool.tile([128, B_ROWS * WP], mybir.dt.bfloat16)
            x_t = {}
            for img in range(IMG_PER_CORE):
                for ci in range(2):
                    if (ci, img) == (0, 0):
                        continue
                    x_t[ci, img] = x_pool.tile(
                        [128, IMG], mybir.dt.bfloat16,
                        tag=f"x_{ci}_{img}", name=f"x_{ci}_{img}",
                    )

            def x_dma(eng, ci, img):
                eng.dma_start(
                    x_t[ci, img][:, :],
                    x_h.ap()[ci][:, img * IMG:(img + 1) * IMG],
                )

            b_sb = const_pool.tile([128, 2], mybir.dt.float32)

            # Two issue streams in first-needed order: DMA queues are FIFO,
            # so earlier transfers drain at full bandwidth before later
            # ones start, instead of fair-sharing with not-yet-needed data.
            nc.sync.dma_start(x00a[:, :], x_h.ap()[0][:, 0:A_ROWS * WP])
            w_dma(nc.scalar, 0, 0)
            nc.sync.dma_start(x00b[:, :],
                              x_h.ap()[0][:, 32 * WP:(32 + B_ROWS) * WP])
            w_dma(nc.scalar, 1, 0)
            x_dma(nc.sync, 1, 0)
            w_dma(nc.scalar, 0, 1)
            x_dma(nc.sync, 0, 1)
            w_dma(nc.scalar, 1, 1)
            x_dma(nc.sync, 1, 1)
            nc.scalar.dma_start(b_sb[:, :], b_h.ap())

            x_sb = {
                k: t[:, :].rearrange("p (r c) -> p r c", c=WP)
                for k, t in x_t.items()
            }
            x00a_v = x00a[:, :].rearrange("p (r c) -> p r c", c=WP)
            x00b_v = x00b[:, :].rearrange("p (r c) -> p r c", c=WP)

            def rhs_ap(ci, img, r0, rows, kh, kw):
                if (ci, img) == (0, 0):
                    if r0 + kh + rows <= A_ROWS:
                        return x00a_v[:, r0 + kh:r0 + kh + rows, kw:kw + W]
                    return x00b_v[
                        :, r0 - 32 + kh:r0 - 32 + kh + rows, kw:kw + W
                    ]
                return x_sb[ci, img][:, r0 + kh:r0 + kh + rows, kw:kw + W]

            def mm(ps, ci, co, img, r0, rows, start, stop):
                for k in range(9):
                    kh, kw = divmod(k, 3)
                    nc.tensor.matmul(
                        ps[:, :],
                        w_sb[ci, co][:, k * 128:(k + 1) * 128],
                        rhs_ap(ci, img, r0, rows, kh, kw),
                        start=start and k == 0,
                        stop=stop and k == 8,
                    )

            def epilogue(ps, co, img, r0, rows):
                n = rows * W
                ot = out_pool.tile([128, CHUNK], mybir.dt.int32, tag="ot")
                nc.vector.tensor_scalar_add(
                    ot[:, :n], ps[:, :], b_sb[:, co:co + 1]
                )
                dst = y_h.ap()[img, co].rearrange("p h w -> p (h w)")[
                    :, r0 * W:r0 * W + n
                ]
                nc.sync.dma_start(dst, ot[:, :n])

            # First plane: sweep ci=0 over the first 4 chunks before any
            # ci=1 matmul, so the PE only gates on the first x and w
            # transfers (w00+x00) instead of all four.
            HEAD = 4
            head_ps = []
            for pc in range(HEAD):
                ps = psum_pool.tile([128, CHUNK], mybir.dt.float32, tag="ps",
                                    name=f"ps_h{pc}")
                head_ps.append(ps)
                mm(ps, 0, 0, 0, pc * ROWS_PER_CHUNK, ROWS_PER_CHUNK,
                   start=True, stop=False)
            for pc in range(HEAD):
                mm(head_ps[pc], 1, 0, 0, pc * ROWS_PER_CHUNK, ROWS_PER_CHUNK,
                   start=False, stop=True)
                epilogue(head_ps[pc], 0, 0, pc * ROWS_PER_CHUNK,
                         ROWS_PER_CHUNK)

            # chunk row-splits per (img, co) plane; the globally last chunk
            # is split [6, 2] so the final PSUM->SBUF->HBM drain is short
            for img in range(IMG_PER_CORE):
                for co in range(2):
                    if img == 0 and co == 0:
                        chunks = [(pc * ROWS_PER_CHUNK, ROWS_PER_CHUNK)
                                  for pc in range(HEAD, N_CHUNKS)]
                    elif img == IMG_PER_CORE - 1 and co == 1:
                        chunks = [(pc * ROWS_PER_CHUNK, ROWS_PER_CHUNK)
                                  for pc in range(N_CHUNKS - 1)]
                        chunks += [(48, 6), (54, 2)]
                    else:
                        chunks = [(pc * ROWS_PER_CHUNK, ROWS_PER_CHUNK)
                                  for pc in range(N_CHUNKS)]
                    for r0, rows in chunks:
                        ps = psum_pool.tile([128, CHUNK], mybir.dt.float32,
                                            tag="ps", name=f"ps_{img}_{co}_{r0}")
                        mm(ps[:, :rows * W], 0, co, img, r0, rows,
                           start=True, stop=False)
                        mm(ps[:, :rows * W], 1, co, img, r0, rows,
                           start=False, stop=True)
                        epilogue(ps[:, :rows * W], co, img, r0, rows)

    nc.compile()
    return nc


_NC = None
LAST_RESULT = None  # BassKernelResults of the most recent run (for harnesses)


def kernel(x_int: np.ndarray, weight_int: np.ndarray, bias_int: np.ndarray):
    from concourse.bass_utils import run_bass_kernel_spmd

    global _NC, LAST_RESULT
    if _NC is None:
        _NC = _build_program()
    nc = _NC

    x_int = np.asarray(x_int)
    weight_int = np.asarray(weight_int)
    bias_int = np.asarray(bias_int)

    # x: pad to 58x58, cast to bf16, split channels into two 128-partition
    # chunks: per core [ci_chunk, 128, img, IMG].
    x_pad = np.zeros((B, C, HP, WP), dtype=_BF16)
    x_pad[:, :, 1:57, 1:57] = x_int.astype(_BF16)
    x_flat = x_pad.reshape(B, 2, 128, IMG)

    # w[co,ci,kh,kw] -> [ci_part, ci_chunk, co_chunk, k, co_part]
    w_t = (
        weight_int.astype(_BF16)
        .reshape(2, 128, 2, 128, 9)          # [co_c, co_p, ci_c, ci_p, k]
        .transpose(3, 2, 0, 4, 1)            # [ci_p, ci_c, co_c, k, co_p]
        .reshape(128, 2 * 2 * 9 * 128)
    )
    w_t = np.ascontiguousarray(w_t)
    b_t = np.ascontiguousarray(
        bias_int.astype(np.float32).reshape(2, 128).T
    )

    in_maps = []
    for c in range(N_CORES):
        xc = np.ascontiguousarray(
            x_flat[c * IMG_PER_CORE:(c + 1) * IMG_PER_CORE].transpose(1, 2, 0, 3)
        )  # [ci_chunk, 128, img, IMG]
        in_maps.append(
            {
                "x": xc.reshape(2, 128, IMG_PER_CORE * IMG),
                "w": w_t,
                "b": b_t,
            }
        )

    res = run_bass_kernel_spmd(nc, in_maps, core_ids=list(range(N_CORES)))
    LAST_RESULT = res

    y = np.empty((B, C, H, W), dtype=np.int32)
    for c in range(N_CORES):
        yc = res.results[c]["y"]  # [img, co_chunk, 128, H, W]
        for img in range(IMG_PER_CORE):
            y[c * IMG_PER_CORE + img] = yc[img].reshape(C, H, W)
    return y


# revision 15
# speedup vs baseline: 1.0252x; 1.0252x over previous
"""Int32 3x3 conv2d (stride 1, pad 1) as bf16 matmuls on 8 TRN2 cores.

Problem: x[16,256,56,56] (*) w[256,256,3,3] + b[256] -> y[16,256,56,56],
all int32, values in [0,127).

Trick: values 0..126 are exactly representable in bf16, every product is
an integer < 2^14, and every accumulation stays < 2^24, so a bf16 matmul
with fp32 PSUM accumulation produces bit-exact integer results.

Layout: each image is zero-padded to 58x58. The 3x3 conv becomes 9
shifted [Cin,Cout]^T @ [Cin,pixels] matmuls accumulated in PSUM; pixel
tiles are 8 output rows x 56 cols = 448 columns (one PSUM bank), read
from the padded image through a strided access pattern so only valid
pixels are computed.

Inputs are packed into 5 bf16 HBM tensors ordered by first use (x row
slab + the w slice needed at the same time), so the critical first
transfers have large per-partition DMA descriptors and a single
dependency unit each.

Sharding: data-parallel over batch, 2 images per core; weights replicated.
"""

import numpy as np
import ml_dtypes

B, C, H, W = 16, 256, 56, 56
HP, WP = H + 2, W + 2          # 58, 58 padded
IMG = HP * WP                  # 3364 flat padded image
N_CORES = 8
IMG_PER_CORE = B // N_CORES    # 2
ROWS_PER_CHUNK = 8
CHUNK = ROWS_PER_CHUNK * W     # 448 valid pixels, fits one PSUM bank
N_CHUNKS = H // ROWS_PER_CHUNK  # 7
N_WARM = 10                    # matmuls to flip the HAM clock gate and
                               # bridge the input-DMA window
A_ROWS = 34                    # x(0,0) slab A: padded rows 0..33
B_ROWS = HP - 32               # x(0,0) slab B: padded rows 32..57
WCOLS = 9 * 128                # one (ci_chunk, co_chunk) weight slice

# packed input tensors: (name, x-columns, has-w-slice)
IN_SPECS = {
    "in0": A_ROWS * WP,        # x00a + w(0,0)
    "in1": IMG,                # x10  + w(1,0)
    "in2": B_ROWS * WP,        # x00b + w(0,1)
    "in3": IMG,                # x01  + w(1,1)
    "in4": IMG,                # x11  (no w)
}

_BF16 = ml_dtypes.bfloat16


def _build_program():
    import concourse.bass as bass
    import concourse.mybir as mybir
    from concourse import bacc
    from concourse.tile import TileContext

    nc = bacc.Bacc("TRN2", target_bir_lowering=False, debug=False)

    in_h = {
        name: nc.dram_tensor(
            name, [128, xc + (WCOLS if name != "in4" else 0)],
            mybir.dt.bfloat16, kind="ExternalInput",
        )
        for name, xc in IN_SPECS.items()
    }
    b_h = nc.dram_tensor("b", [128, 2], mybir.dt.float32, kind="ExternalInput")
    y_h = nc.dram_tensor(
        "y", [IMG_PER_CORE, 2, 128, H, W], mybir.dt.int32, kind="ExternalOutput"
    )

    with TileContext(nc) as tc:
        with (
            tc.tile_pool(name="const", bufs=1) as const_pool,
            tc.tile_pool(name="xin", bufs=1) as x_pool,
            tc.tile_pool(name="psum", bufs=5, space="PSUM") as psum_pool,
            tc.tile_pool(name="warm", bufs=1, space="PSUM") as warm_pool,
            tc.tile_pool(name="outs", bufs=2) as out_pool,
        ):
            # PE warm-up: junk matmuls on a zeroed tile while the input
            # DMAs land, so the HAM clock gate is at 8/8 (2.4 GHz) when
            # the real matmuls start.
            wz = const_pool.tile([128, 128 + CHUNK], mybir.dt.bfloat16)
            nc.vector.memset(wz[:, :], 0.0)
            wps = warm_pool.tile([128, CHUNK], mybir.dt.float32)
            for i in range(N_WARM):
                nc.tensor.matmul(
                    wps[:, :], wz[:, 0:128], wz[:, 128:128 + CHUNK],
                    start=True, stop=True,
                )

            in_sb = {
                name: x_pool.tile(
                    [128, int(in_h[name].shape[1])], mybir.dt.bfloat16,
                    tag=name, name=f"t_{name}",
                )
                for name in IN_SPECS
            }
            b_sb = const_pool.tile([128, 2], mybir.dt.float32)

            # Two issue streams in first-needed order: DMA queues are FIFO,
            # so earlier transfers drain at full bandwidth before later
            # ones start, instead of fair-sharing with not-yet-needed data.
            nc.sync.dma_start(in_sb["in0"][:, :], in_h["in0"].ap())
            nc.scalar.dma_start(in_sb["in1"][:, :], in_h["in1"].ap())
            nc.sync.dma_start(in_sb["in2"][:, :], in_h["in2"].ap())
            nc.scalar.dma_start(in_sb["in3"][:, :], in_h["in3"].ap())
            nc.sync.dma_start(in_sb["in4"][:, :], in_h["in4"].ap())
            nc.scalar.dma_start(b_sb[:, :], b_h.ap())

            # weight slice views: (ci, co) -> [128, 9*128] region
            w_sb = {
                (0, 0): in_sb["in0"][:, A_ROWS * WP:],
                (1, 0): in_sb["in1"][:, IMG:],
                (0, 1): in_sb["in2"][:, B_ROWS * WP:],
                (1, 1): in_sb["in3"][:, IMG:],
            }
            # x views as [128, rows, 58]
            x00a_v = in_sb["in0"][:, :A_ROWS * WP].rearrange(
                "p (r c) -> p r c", c=WP)
            x00b_v = in_sb["in2"][:, :B_ROWS * WP].rearrange(
                "p (r c) -> p r c", c=WP)
            x_sb = {
                (1, 0): in_sb["in1"][:, :IMG].rearrange("p (r c) -> p r c", c=WP),
                (0, 1): in_sb["in3"][:, :IMG].rearrange("p (r c) -> p r c", c=WP),
                (1, 1): in_sb["in4"][:, :IMG].rearrange("p (r c) -> p r c", c=WP),
            }

            def rhs_ap(ci, img, r0, rows, kh, kw):
                if (ci, img) == (0, 0):
                    if r0 + kh + rows <= A_ROWS:
                        return x00a_v[:, r0 + kh:r0 + kh + rows, kw:kw + W]
                    return x00b_v[
                        :, r0 - 32 + kh:r0 - 32 + kh + rows, kw:kw + W
                    ]
                return x_sb[ci, img][:, r0 + kh:r0 + kh + rows, kw:kw + W]

            def mm(ps, ci, co, img, r0, rows, start, stop):
                for k in range(9):
                    kh, kw = divmod(k, 3)
                    nc.tensor.matmul(
                        ps[:, :],
                        w_sb[ci, co][:, k * 128:(k + 1) * 128],
                        rhs_ap(ci, img, r0, rows, kh, kw),
                        start=start and k == 0,
                        stop=stop and k == 8,
                    )

            def epilogue(ps, co, img, r0, rows):
                n = rows * W
                ot = out_pool.tile([128, CHUNK], mybir.dt.int32, tag="ot")
                nc.vector.tensor_scalar_add(
                    ot[:, :n], ps[:, :], b_sb[:, co:co + 1]
                )
                dst = y_h.ap()[img, co].rearrange("p h w -> p (h w)")[
                    :, r0 * W:r0 * W + n
                ]
                nc.sync.dma_start(dst, ot[:, :n])

            # First plane: sweep ci=0 over the first 4 chunks before any
            # ci=1 matmul, so the PE only gates on the first packed
            # transfer (x00a + w00).
            HEAD = 4
            head_ps = []
            for pc in range(HEAD):
                ps = psum_pool.tile([128, CHUNK], mybir.dt.float32, tag="ps",
                                    name=f"ps_h{pc}")
                head_ps.append(ps)
                mm(ps, 0, 0, 0, pc * ROWS_PER_CHUNK, ROWS_PER_CHUNK,
                   start=True, stop=False)
            for pc in range(HEAD):
                mm(head_ps[pc], 1, 0, 0, pc * ROWS_PER_CHUNK, ROWS_PER_CHUNK,
                   start=False, stop=True)
                epilogue(head_ps[pc], 0, 0, pc * ROWS_PER_CHUNK,
                         ROWS_PER_CHUNK)

            # chunk row-splits per (img, co) plane; the globally last chunk
            # is split [6, 2] so the final PSUM->SBUF->HBM drain is short
            for img in range(IMG_PER_CORE):
                for co in range(2):
                    if img == 0 and co == 0:
                        chunks = [(pc * ROWS_PER_CHUNK, ROWS_PER_CHUNK)
                                  for pc in range(HEAD, N_CHUNKS)]
                    elif img == IMG_PER_CORE - 1 and co == 1:
                        chunks = [(pc * ROWS_PER_CHUNK, ROWS_PER_CHUNK)
                                  for pc in range(N_CHUNKS - 1)]
                        chunks += [(48, 6), (54, 2)]
                    else:
                        chunks = [(pc * ROWS_PER_CHUNK, ROWS_PER_CHUNK)
                                  for pc in range(N_CHUNKS)]
                    for r0, rows in chunks:
                        ps = psum_pool.tile([128, CHUNK], mybir.dt.float32,
                                            tag="ps", name=f"ps_{img}_{co}_{r0}")
                        mm(ps[:, :rows * W], 0, co, img, r0, rows,
                           start=True, stop=False)
                        mm(ps[:, :rows * W], 1, co, img, r0, rows,
                           start=False, stop=True)
                        epilogue(ps[:, :rows * W], co, img, r0, rows)

    nc.compile()
    return nc


_NC = None
LAST_RESULT = None  # BassKernelResults of the most recent run (for harnesses)


def kernel(x_int: np.ndarray, weight_int: np.ndarray, bias_int: np.ndarray):
    from concourse.bass_utils import run_bass_kernel_spmd

    global _NC, LAST_RESULT
    if _NC is None:
        _NC = _build_program()
    nc = _NC

    x_int = np.asarray(x_int)
    weight_int = np.asarray(weight_int)
    bias_int = np.asarray(bias_int)

    # x: pad to 58x58, cast to bf16, split channels into two 128-partition
    # chunks: x_flat[b, ci_chunk, 128, IMG]
    x_pad = np.zeros((B, C, HP, WP), dtype=_BF16)
    x_pad[:, :, 1:57, 1:57] = x_int.astype(_BF16)
    x_flat = x_pad.reshape(B, 2, 128, IMG)

    # w[co,ci,kh,kw] -> [ci_part, (ci_chunk, co_chunk, k, co_part)]
    w_t = (
        weight_int.astype(_BF16)
        .reshape(2, 128, 2, 128, 9)          # [co_c, co_p, ci_c, ci_p, k]
        .transpose(3, 2, 0, 4, 1)            # [ci_p, ci_c, co_c, k, co_p]
        .reshape(128, 2 * 2 * 9 * 128)
    )

    def w_slice(ci, co):
        s = (ci * 2 + co) * WCOLS
        return w_t[:, s:s + WCOLS]

    b_t = np.ascontiguousarray(
        bias_int.astype(np.float32).reshape(2, 128).T
    )

    in_maps = []
    for c in range(N_CORES):
        xs = x_flat[c * IMG_PER_CORE:(c + 1) * IMG_PER_CORE]  # [img, ci_c, 128, IMG]
        x00, x10 = xs[0, 0], xs[0, 1]
        x01, x11 = xs[1, 0], xs[1, 1]
        in_maps.append(
            {
                "in0": np.ascontiguousarray(np.concatenate(
                    [x00[:, :A_ROWS * WP], w_slice(0, 0)], axis=1)),
                "in1": np.ascontiguousarray(np.concatenate(
                    [x10, w_slice(1, 0)], axis=1)),
                "in2": np.ascontiguousarray(np.concatenate(
                    [x00[:, 32 * WP:], w_slice(0, 1)], axis=1)),
                "in3": np.ascontiguousarray(np.concatenate(
                    [x01, w_slice(1, 1)], axis=1)),
                "in4": np.ascontiguousarray(x11),
                "b": b_t,
            }
        )

    res = run_bass_kernel_spmd(nc, in_maps, core_ids=list(range(N_CORES)))
    LAST_RESULT = res

    y = np.empty((B, C, H, W), dtype=np.int32)
    for c in range(N_CORES):
        yc = res.results[c]["y"]  # [img, co_chunk, 128, H, W]
        for img in range(IMG_PER_CORE):
            y[c * IMG_PER_CORE + img] = yc[img].reshape(C, H, W)
    return y


# revision 16
# speedup vs baseline: 1.0317x; 1.0064x over previous
"""Int32 3x3 conv2d (stride 1, pad 1) as bf16 matmuls on 8 TRN2 cores.

Problem: x[16,256,56,56] (*) w[256,256,3,3] + b[256] -> y[16,256,56,56],
all int32, values in [0,127).

Trick: values 0..126 are exactly representable in bf16, every product is
an integer < 2^14, and every accumulation stays < 2^24, so a bf16 matmul
with fp32 PSUM accumulation produces bit-exact integer results.

Layout: each image is zero-padded to 58x58. The 3x3 conv becomes 9
shifted [Cin,Cout]^T @ [Cin,pixels] matmuls accumulated in PSUM; pixel
tiles are 8 output rows x 56 cols = 448 columns (one PSUM bank), read
from the padded image through a strided access pattern so only valid
pixels are computed.

Inputs are packed into 5 bf16 HBM tensors ordered by first use (x row
slab + the w slice needed at the same time), so the critical first
transfers have large per-partition DMA descriptors and a single
dependency unit each.

Sharding: data-parallel over batch, 2 images per core; weights replicated.
"""

import numpy as np
import ml_dtypes

B, C, H, W = 16, 256, 56, 56
HP, WP = H + 2, W + 2          # 58, 58 padded
IMG = HP * WP                  # 3364 flat padded image
N_CORES = 8
IMG_PER_CORE = B // N_CORES    # 2
ROWS_PER_CHUNK = 8
CHUNK = ROWS_PER_CHUNK * W     # 448 valid pixels, fits one PSUM bank
N_CHUNKS = H // ROWS_PER_CHUNK  # 7
N_WARM = 9                     # matmuls to flip the HAM clock gate and
                               # bridge the input-DMA window
A_ROWS = 34                    # x(0,0) slab A: padded rows 0..33
B_ROWS = HP - 32               # x(0,0) slab B: padded rows 32..57
WCOLS = 9 * 128                # one (ci_chunk, co_chunk) weight slice

# packed input tensors: (name, x-columns, has-w-slice)
IN_SPECS = {
    "in0": A_ROWS * WP,        # x00a + w(0,0)
    "in1": IMG,                # x10  + w(1,0)
    "in2": B_ROWS * WP,        # x00b + w(0,1)
    "in3": IMG,                # x01  + w(1,1)
    "in4": IMG,                # x11  (no w)
}

_BF16 = ml_dtypes.bfloat16


def _build_program():
    import concourse.bass as bass
    import concourse.mybir as mybir
    from concourse import bacc
    from concourse.tile import TileContext

    nc = bacc.Bacc("TRN2", target_bir_lowering=False, debug=False)

    in_h = {
        name: nc.dram_tensor(
            name, [128, xc + (WCOLS if name != "in4" else 0)],
            mybir.dt.bfloat16, kind="ExternalInput",
        )
        for name, xc in IN_SPECS.items()
    }
    b_h = nc.dram_tensor("b", [128, 2], mybir.dt.float32, kind="ExternalInput")
    y_h = nc.dram_tensor(
        "y", [IMG_PER_CORE, 2, 128, H, W], mybir.dt.int32, kind="ExternalOutput"
    )

    with TileContext(nc) as tc:
        with (
            tc.tile_pool(name="const", bufs=1) as const_pool,
            tc.tile_pool(name="xin", bufs=1) as x_pool,
            tc.tile_pool(name="psum", bufs=5, space="PSUM") as psum_pool,
            tc.tile_pool(name="warm", bufs=1, space="PSUM") as warm_pool,
            tc.tile_pool(name="outs", bufs=2) as out_pool,
        ):
            # PE warm-up: junk matmuls on a zeroed tile while the input
            # DMAs land, so the HAM clock gate is at 8/8 (2.4 GHz) when
            # the real matmuls start.
            wz = const_pool.tile([128, 128 + CHUNK], mybir.dt.bfloat16)
            nc.vector.memset(wz[:, :], 0.0)
            wps = warm_pool.tile([128, CHUNK], mybir.dt.float32)
            for i in range(N_WARM):
                nc.tensor.matmul(
                    wps[:, :], wz[:, 0:128], wz[:, 128:128 + CHUNK],
                    start=True, stop=True,
                )

            in_sb = {
                name: x_pool.tile(
                    [128, int(in_h[name].shape[1])], mybir.dt.bfloat16,
                    tag=name, name=f"t_{name}",
                )
                for name in IN_SPECS
            }
            b_sb = const_pool.tile([128, 2], mybir.dt.float32)

            # Two issue streams in first-needed order: DMA queues are FIFO,
            # so earlier transfers drain at full bandwidth before later
            # ones start, instead of fair-sharing with not-yet-needed data.
            nc.scalar.dma_start(b_sb[:, :], b_h.ap())
            nc.sync.dma_start(in_sb["in0"][:, :], in_h["in0"].ap())
            nc.sync.dma_start(in_sb["in1"][:, :], in_h["in1"].ap())
            nc.sync.dma_start(in_sb["in2"][:, :], in_h["in2"].ap())
            nc.sync.dma_start(in_sb["in3"][:, :], in_h["in3"].ap())
            nc.sync.dma_start(in_sb["in4"][:, :], in_h["in4"].ap())

            # weight slice views: (ci, co) -> [128, 9*128] region
            w_sb = {
                (0, 0): in_sb["in0"][:, A_ROWS * WP:],
                (1, 0): in_sb["in1"][:, IMG:],
                (0, 1): in_sb["in2"][:, B_ROWS * WP:],
                (1, 1): in_sb["in3"][:, IMG:],
            }
            # x views as [128, rows, 58]
            x00a_v = in_sb["in0"][:, :A_ROWS * WP].rearrange(
                "p (r c) -> p r c", c=WP)
            x00b_v = in_sb["in2"][:, :B_ROWS * WP].rearrange(
                "p (r c) -> p r c", c=WP)
            x_sb = {
                (1, 0): in_sb["in1"][:, :IMG].rearrange("p (r c) -> p r c", c=WP),
                (0, 1): in_sb["in3"][:, :IMG].rearrange("p (r c) -> p r c", c=WP),
                (1, 1): in_sb["in4"][:, :IMG].rearrange("p (r c) -> p r c", c=WP),
            }

            def rhs_ap(ci, img, r0, rows, kh, kw):
                if (ci, img) == (0, 0):
                    if r0 + kh + rows <= A_ROWS:
                        return x00a_v[:, r0 + kh:r0 + kh + rows, kw:kw + W]
                    return x00b_v[
                        :, r0 - 32 + kh:r0 - 32 + kh + rows, kw:kw + W
                    ]
                return x_sb[ci, img][:, r0 + kh:r0 + kh + rows, kw:kw + W]

            def mm(ps, ci, co, img, r0, rows, start, stop):
                for k in range(9):
                    kh, kw = divmod(k, 3)
                    nc.tensor.matmul(
                        ps[:, :],
                        w_sb[ci, co][:, k * 128:(k + 1) * 128],
                        rhs_ap(ci, img, r0, rows, kh, kw),
                        start=start and k == 0,
                        stop=stop and k == 8,
                    )

            def epilogue(ps, co, img, r0, rows):
                n = rows * W
                ot = out_pool.tile([128, CHUNK], mybir.dt.int32, tag="ot")
                nc.vector.tensor_scalar_add(
                    ot[:, :n], ps[:, :], b_sb[:, co:co + 1]
                )
                dst = y_h.ap()[img, co].rearrange("p h w -> p (h w)")[
                    :, r0 * W:r0 * W + n
                ]
                nc.sync.dma_start(dst, ot[:, :n])

            # First plane: sweep ci=0 over the first 4 chunks before any
            # ci=1 matmul, so the PE only gates on the first packed
            # transfer (x00a + w00).
            HEAD = 4
            head_ps = []
            for pc in range(HEAD):
                ps = psum_pool.tile([128, CHUNK], mybir.dt.float32, tag="ps",
                                    name=f"ps_h{pc}")
                head_ps.append(ps)
                mm(ps, 0, 0, 0, pc * ROWS_PER_CHUNK, ROWS_PER_CHUNK,
                   start=True, stop=False)
            for pc in range(HEAD):
                mm(head_ps[pc], 1, 0, 0, pc * ROWS_PER_CHUNK, ROWS_PER_CHUNK,
                   start=False, stop=True)
                epilogue(head_ps[pc], 0, 0, pc * ROWS_PER_CHUNK,
                         ROWS_PER_CHUNK)

            # chunk row-splits per (img, co) plane; the globally last chunk
            # is split [6, 2] so the final PSUM->SBUF->HBM drain is short
            for img in range(IMG_PER_CORE):
                for co in range(2):
                    if img == 0 and co == 0:
                        chunks = [(pc * ROWS_PER_CHUNK, ROWS_PER_CHUNK)
                                  for pc in range(HEAD, N_CHUNKS)]
                    elif img == IMG_PER_CORE - 1 and co == 1:
                        chunks = [(pc * ROWS_PER_CHUNK, ROWS_PER_CHUNK)
                                  for pc in range(N_CHUNKS - 1)]
                        chunks += [(48, 6), (54, 2)]
                    else:
                        chunks = [(pc * ROWS_PER_CHUNK, ROWS_PER_CHUNK)
                                  for pc in range(N_CHUNKS)]
                    for r0, rows in chunks:
                        ps = psum_pool.tile([128, CHUNK], mybir.dt.float32,
                                            tag="ps", name=f"ps_{img}_{co}_{r0}")
                        mm(ps[:, :rows * W], 0, co, img, r0, rows,
                           start=True, stop=False)
                        mm(ps[:, :rows * W], 1, co, img, r0, rows,
                           start=False, stop=True)
                        epilogue(ps[:, :rows * W], co, img, r0, rows)

    nc.compile()
    return nc


_NC = None
LAST_RESULT = None  # BassKernelResults of the most recent run (for harnesses)


def kernel(x_int: np.ndarray, weight_int: np.ndarray, bias_int: np.ndarray):
    from concourse.bass_utils import run_bass_kernel_spmd

    global _NC, LAST_RESULT
    if _NC is None:
        _NC = _build_program()
    nc = _NC

    x_int = np.asarray(x_int)
    weight_int = np.asarray(weight_int)
    bias_int = np.asarray(bias_int)

    # x: pad to 58x58, cast to bf16, split channels into two 128-partition
    # chunks: x_flat[b, ci_chunk, 128, IMG]
    x_pad = np.zeros((B, C, HP, WP), dtype=_BF16)
    x_pad[:, :, 1:57, 1:57] = x_int.astype(_BF16)
    x_flat = x_pad.reshape(B, 2, 128, IMG)

    # w[co,ci,kh,kw] -> [ci_part, (ci_chunk, co_chunk, k, co_part)]
    w_t = (
        weight_int.astype(_BF16)
        .reshape(2, 128, 2, 128, 9)          # [co_c, co_p, ci_c, ci_p, k]
        .transpose(3, 2, 0, 4, 1)            # [ci_p, ci_c, co_c, k, co_p]
        .reshape(128, 2 * 2 * 9 * 128)
    )

    def w_slice(ci, co):
        s = (ci * 2 + co) * WCOLS
        return w_t[:, s:s + WCOLS]

    b_t = np.ascontiguousarray(
        bias_int.astype(np.float32).reshape(2, 128).T
    )

    in_maps = []
    for c in range(N_CORES):
        xs = x_flat[c * IMG_PER_CORE:(c + 1) * IMG_PER_CORE]  # [img, ci_c, 128, IMG]
        x00, x10 = xs[0, 0], xs[0, 1]
        x01, x11 = xs[1, 0], xs[1, 1]
        in_maps.append(
            {
                "in0": np.ascontiguousarray(np.concatenate(
                    [x00[:, :A_ROWS * WP], w_slice(0, 0)], axis=1)),
                "in1": np.ascontiguousarray(np.concatenate(
                    [x10, w_slice(1, 0)], axis=1)),
                "in2": np.ascontiguousarray(np.concatenate(
                    [x00[:, 32 * WP:], w_slice(0, 1)], axis=1)),
                "in3": np.ascontiguousarray(np.concatenate(
                    [x01, w_slice(1, 1)], axis=1)),
                "in4": np.ascontiguousarray(x11),
                "b": b_t,
            }
        )

    res = run_bass_kernel_spmd(nc, in_maps, core_ids=list(range(N_CORES)))
    LAST_RESULT = res

    y = np.empty((B, C, H, W), dtype=np.int32)
    for c in range(N_CORES):
        yc = res.results[c]["y"]  # [img, co_chunk, 128, H, W]
        for img in range(IMG_PER_CORE):
            y[c * IMG_PER_CORE + img] = yc[img].reshape(C, H, W)
    return y


# revision 17
# speedup vs baseline: 1.0412x; 1.0092x over previous
"""Int32 3x3 conv2d (stride 1, pad 1) as bf16 matmuls on 8 TRN2 cores.

Problem: x[16,256,56,56] (*) w[256,256,3,3] + b[256] -> y[16,256,56,56],
all int32, values in [0,127).

Trick: values 0..126 are exactly representable in bf16, every product is
an integer < 2^14, and every accumulation stays < 2^24, so a bf16 matmul
with fp32 PSUM accumulation produces bit-exact integer results.

Layout: each image is zero-padded to 58x58. The 3x3 conv becomes 9
shifted [Cin,Cout]^T @ [Cin,pixels] matmuls accumulated in PSUM; pixel
tiles are 8 output rows x 56 cols = 448 columns (one PSUM bank), read
from the padded image through a strided access pattern so only valid
pixels are computed. The kw=1 taps read a host-prepared copy of the
image shifted left by one element, keeping every matmul's moving
operand 4-byte aligned (a 2-byte-misaligned base costs ~7 ns/matmul).

Inputs are packed into bf16 HBM tensors ordered by first use (x row
slab + the w slice needed at the same time), so the critical first
transfers have large per-partition DMA descriptors and a single
dependency unit each.

Sharding: data-parallel over batch, 2 images per core; weights replicated.
"""

import numpy as np
import ml_dtypes

B, C, H, W = 16, 256, 56, 56
HP, WP = H + 2, W + 2          # 58, 58 padded
IMG = HP * WP                  # 3364 flat padded image
N_CORES = 8
IMG_PER_CORE = B // N_CORES    # 2
ROWS_PER_CHUNK = 8
CHUNK = ROWS_PER_CHUNK * W     # 448 valid pixels, fits one PSUM bank
N_CHUNKS = H // ROWS_PER_CHUNK  # 7
N_WARM = 9                     # matmuls to flip the HAM clock gate and
                               # bridge the input-DMA window
A_ROWS = 34                    # x(0,0) slab A: padded rows 0..33
B_ROWS = HP - 32               # x(0,0) slab B: padded rows 32..57
WCOLS = 9 * 128                # one (ci_chunk, co_chunk) weight slice

# packed input tensors: name -> (x-columns, carries-w-slice)
IN_SPECS = {
    "in0": (A_ROWS * WP, True),   # x00a       + w(0,0)
    "in5": (IMG, False),          # x00 shifted
    "in1": (IMG, True),           # x10        + w(1,0)
    "in6": (IMG, False),          # x10 shifted
    "in2": (B_ROWS * WP, True),   # x00b       + w(0,1)
    "in3": (IMG, True),           # x01        + w(1,1)
    "in4": (IMG, False),          # x11
    "in7": (IMG, False),          # x01 shifted
    "in8": (IMG, False),          # x11 shifted
}
K_ALIGNED = [0, 2, 3, 5, 6, 8]   # kw in {0, 2}: 4B-aligned in the plain copy
K_SHIFTED = [1, 4, 7]            # kw == 1: read the shifted copy at kw=0

_BF16 = ml_dtypes.bfloat16


def _build_program():
    import concourse.bass as bass
    import concourse.mybir as mybir
    from concourse import bacc
    from concourse.tile import TileContext

    nc = bacc.Bacc("TRN2", target_bir_lowering=False, debug=False)

    in_h = {
        name: nc.dram_tensor(
            name, [128, xc + (WCOLS if has_w else 0)],
            mybir.dt.bfloat16, kind="ExternalInput",
        )
        for name, (xc, has_w) in IN_SPECS.items()
    }
    b_h = nc.dram_tensor("b", [128, 2], mybir.dt.float32, kind="ExternalInput")
    y_h = nc.dram_tensor(
        "y", [IMG_PER_CORE, 2, 128, H, W], mybir.dt.int32, kind="ExternalOutput"
    )

    with TileContext(nc) as tc:
        with (
            tc.tile_pool(name="const", bufs=1) as const_pool,
            tc.tile_pool(name="xin", bufs=1) as x_pool,
            tc.tile_pool(name="psum", bufs=5, space="PSUM") as psum_pool,
            tc.tile_pool(name="warm", bufs=1, space="PSUM") as warm_pool,
            tc.tile_pool(name="outs", bufs=2) as out_pool,
        ):
            # PE warm-up: junk matmuls on a zeroed tile while the input
            # DMAs land, so the HAM clock gate is at 8/8 (2.4 GHz) when
            # the real matmuls start.
            wz = const_pool.tile([128, 128 + CHUNK], mybir.dt.bfloat16)
            nc.vector.memset(wz[:, :], 0.0)
            wps = warm_pool.tile([128, CHUNK], mybir.dt.float32)
            for i in range(N_WARM):
                nc.tensor.matmul(
                    wps[:, :], wz[:, 0:128], wz[:, 128:128 + CHUNK],
                    start=True, stop=True,
                )

            in_sb = {
                name: x_pool.tile(
                    [128, int(in_h[name].shape[1])], mybir.dt.bfloat16,
                    tag=name, name=f"t_{name}",
                )
                for name in IN_SPECS
            }
            b_sb = const_pool.tile([128, 2], mybir.dt.float32)

            # One input issue stream in first-needed order: DMA queues are
            # FIFO, so earlier transfers drain at full bandwidth before
            # later ones start, instead of fair-sharing with
            # not-yet-needed data.
            nc.scalar.dma_start(b_sb[:, :], b_h.ap())
            for name in IN_SPECS:
                nc.sync.dma_start(in_sb[name][:, :], in_h[name].ap())

            # weight slice views: (ci, co) -> [128, 9*128] region
            w_sb = {
                (0, 0): in_sb["in0"][:, A_ROWS * WP:],
                (1, 0): in_sb["in1"][:, IMG:],
                (0, 1): in_sb["in2"][:, B_ROWS * WP:],
                (1, 1): in_sb["in3"][:, IMG:],
            }

            def xview(name, cols):
                return in_sb[name][:, :cols].rearrange("p (r c) -> p r c", c=WP)

            x00a_v = xview("in0", A_ROWS * WP)
            x00b_v = xview("in2", B_ROWS * WP)
            x_sb = {
                (1, 0): xview("in1", IMG),
                (0, 1): xview("in3", IMG),
                (1, 1): xview("in4", IMG),
            }
            x_shift = {
                (0, 0): xview("in5", IMG),
                (1, 0): xview("in6", IMG),
                (0, 1): xview("in7", IMG),
                (1, 1): xview("in8", IMG),
            }

            def rhs_ap(ci, img, r0, rows, kh, kw):
                r = r0 + kh
                if kw == 1:
                    return x_shift[ci, img][:, r:r + rows, 0:W]
                if (ci, img) == (0, 0):
                    if r + rows <= A_ROWS:
                        return x00a_v[:, r:r + rows, kw:kw + W]
                    return x00b_v[:, r - 32:r - 32 + rows, kw:kw + W]
                return x_sb[ci, img][:, r:r + rows, kw:kw + W]

            def mm(ps, ci, co, img, r0, rows, ks, start, stop):
                for i, k in enumerate(ks):
                    kh, kw = divmod(k, 3)
                    nc.tensor.matmul(
                        ps[:, :],
                        w_sb[ci, co][:, k * 128:(k + 1) * 128],
                        rhs_ap(ci, img, r0, rows, kh, kw),
                        start=start and i == 0,
                        stop=stop and i == len(ks) - 1,
                    )

            def epilogue(ps, co, img, r0, rows):
                n = rows * W
                ot = out_pool.tile([128, CHUNK], mybir.dt.int32, tag="ot")
                nc.vector.tensor_scalar_add(
                    ot[:, :n], ps[:, :], b_sb[:, co:co + 1]
                )
                dst = y_h.ap()[img, co].rearrange("p h w -> p (h w)")[
                    :, r0 * W:r0 * W + n
                ]
                nc.sync.dma_start(dst, ot[:, :n])

            # First plane: sweep ci=0 over the first 4 chunks before any
            # ci=1 matmul, aligned taps before shifted taps, so the PE
            # only gates on the first packed transfer (x00a + w00) and the
            # shifted copy (in5) has time to arrive.
            HEAD = 4
            head_ps = []
            for pc in range(HEAD):
                ps = psum_pool.tile([128, CHUNK], mybir.dt.float32, tag="ps",
                                    name=f"ps_h{pc}")
                head_ps.append(ps)
                mm(ps, 0, 0, 0, pc * ROWS_PER_CHUNK, ROWS_PER_CHUNK,
                   K_ALIGNED, start=True, stop=False)
            for pc in range(HEAD):
                mm(head_ps[pc], 0, 0, 0, pc * ROWS_PER_CHUNK, ROWS_PER_CHUNK,
                   K_SHIFTED, start=False, stop=False)
            for pc in range(HEAD):
                mm(head_ps[pc], 1, 0, 0, pc * ROWS_PER_CHUNK, ROWS_PER_CHUNK,
                   K_ALIGNED + K_SHIFTED, start=False, stop=True)
                epilogue(head_ps[pc], 0, 0, pc * ROWS_PER_CHUNK,
                         ROWS_PER_CHUNK)

            # chunk row-splits per (img, co) plane; the globally last chunk
            # is split [6, 2] so the final PSUM->SBUF->HBM drain is short
            for img in range(IMG_PER_CORE):
                for co in range(2):
                    if img == 0 and co == 0:
                        chunks = [(pc * ROWS_PER_CHUNK, ROWS_PER_CHUNK)
                                  for pc in range(HEAD, N_CHUNKS)]
                    elif img == IMG_PER_CORE - 1 and co == 1:
                        chunks = [(pc * ROWS_PER_CHUNK, ROWS_PER_CHUNK)
                                  for pc in range(N_CHUNKS - 1)]
                        chunks += [(48, 6), (54, 2)]
                    else:
                        chunks = [(pc * ROWS_PER_CHUNK, ROWS_PER_CHUNK)
                                  for pc in range(N_CHUNKS)]
                    for r0, rows in chunks:
                        ps = psum_pool.tile([128, CHUNK], mybir.dt.float32,
                                            tag="ps", name=f"ps_{img}_{co}_{r0}")
                        mm(ps[:, :rows * W], 0, co, img, r0, rows,
                           K_ALIGNED + K_SHIFTED, start=True, stop=False)
                        mm(ps[:, :rows * W], 1, co, img, r0, rows,
                           K_ALIGNED + K_SHIFTED, start=False, stop=True)
                        epilogue(ps[:, :rows * W], co, img, r0, rows)

    nc.compile()
    return nc


_NC = None
LAST_RESULT = None  # BassKernelResults of the most recent run (for harnesses)


def kernel(x_int: np.ndarray, weight_int: np.ndarray, bias_int: np.ndarray):
    from concourse.bass_utils import run_bass_kernel_spmd

    global _NC, LAST_RESULT
    if _NC is None:
        _NC = _build_program()
    nc = _NC

    x_int = np.asarray(x_int)
    weight_int = np.asarray(weight_int)
    bias_int = np.asarray(bias_int)

    # x: pad to 58x58, cast to bf16, split channels into two 128-partition
    # chunks: x_flat[b, ci_chunk, 128, 58, 58]
    x_pad = np.zeros((B, C, HP, WP), dtype=_BF16)
    x_pad[:, :, 1:57, 1:57] = x_int.astype(_BF16)
    x_r = x_pad.reshape(B, 2, 128, HP, WP)
    # left-shift-by-one copy: xs[.., c] = x[.., c+1]
    x_s = np.zeros_like(x_r)
    x_s[..., :WP - 1] = x_r[..., 1:]
    x_flat = x_r.reshape(B, 2, 128, IMG)
    x_sflat = x_s.reshape(B, 2, 128, IMG)

    # w[co,ci,kh,kw] -> [ci_part, (ci_chunk, co_chunk, k, co_part)]
    w_t = (
        weight_int.astype(_BF16)
        .reshape(2, 128, 2, 128, 9)          # [co_c, co_p, ci_c, ci_p, k]
        .transpose(3, 2, 0, 4, 1)            # [ci_p, ci_c, co_c, k, co_p]
        .reshape(128, 2 * 2 * 9 * 128)
    )

    def w_slice(ci, co):
        s = (ci * 2 + co) * WCOLS
        return w_t[:, s:s + WCOLS]

    b_t = np.ascontiguousarray(
        bias_int.astype(np.float32).reshape(2, 128).T
    )

    def cat(*arrs):
        return np.ascontiguousarray(np.concatenate(arrs, axis=1))

    in_maps = []
    for c in range(N_CORES):
        xs = x_flat[c * IMG_PER_CORE:(c + 1) * IMG_PER_CORE]
        ss = x_sflat[c * IMG_PER_CORE:(c + 1) * IMG_PER_CORE]
        in_maps.append(
            {
                "in0": cat(xs[0, 0][:, :A_ROWS * WP], w_slice(0, 0)),
                "in5": np.ascontiguousarray(ss[0, 0]),
                "in1": cat(xs[0, 1], w_slice(1, 0)),
                "in6": np.ascontiguousarray(ss[0, 1]),
                "in2": cat(xs[0, 0][:, 32 * WP:], w_slice(0, 1)),
                "in3": cat(xs[1, 0], w_slice(1, 1)),
                "in4": np.ascontiguousarray(xs[1, 1]),
                "in7": np.ascontiguousarray(ss[1, 0]),
                "in8": np.ascontiguousarray(ss[1, 1]),
                "b": b_t,
            }
        )

    res = run_bass_kernel_spmd(nc, in_maps, core_ids=list(range(N_CORES)))
    LAST_RESULT = res

    y = np.empty((B, C, H, W), dtype=np.int32)
    for c in range(N_CORES):
        yc = res.results[c]["y"]  # [img, co_chunk, 128, H, W]
        for img in range(IMG_PER_CORE):
            y[c * IMG_PER_CORE + img] = yc[img].reshape(C, H, W)
    return y


# revision 18
# speedup vs baseline: 1.0420x; 1.0008x over previous
"""Int32 3x3 conv2d (stride 1, pad 1) as bf16 matmuls on 8 TRN2 cores.

Problem: x[16,256,56,56] (*) w[256,256,3,3] + b[256] -> y[16,256,56,56],
all int32, values in [0,127).

Trick: values 0..126 are exactly representable in bf16, every product is
an integer < 2^14, and every accumulation stays < 2^24, so a bf16 matmul
with fp32 PSUM accumulation produces bit-exact integer results.

Layout: each image is zero-padded to 58x58. The 3x3 conv becomes 9
shifted [Cin,Cout]^T @ [Cin,pixels] matmuls accumulated in PSUM; pixel
tiles are 8 output rows x 56 cols = 448 columns (one PSUM bank), read
from the padded image through a strided access pattern so only valid
pixels are computed. The kw=1 taps read a host-prepared copy of the
image shifted left by one element, keeping every matmul's moving
operand 4-byte aligned (a 2-byte-misaligned base costs ~7 ns/matmul).

Inputs are packed into bf16 HBM tensors ordered by first use (x row
slab + the w slice needed at the same time), so the critical first
transfers have large per-partition DMA descriptors and a single
dependency unit each.

Sharding: data-parallel over batch, 2 images per core; weights replicated.
"""

import numpy as np
import ml_dtypes

B, C, H, W = 16, 256, 56, 56
HP, WP = H + 2, W + 2          # 58, 58 padded
IMG = HP * WP                  # 3364 flat padded image
N_CORES = 8
IMG_PER_CORE = B // N_CORES    # 2
ROWS_PER_CHUNK = 8
CHUNK = ROWS_PER_CHUNK * W     # 448 valid pixels, fits one PSUM bank
N_CHUNKS = H // ROWS_PER_CHUNK  # 7
N_WARM = 34                    # small (N=128) matmuls to flip the HAM clock
                               # gate and bridge the input-DMA window
A_ROWS = 34                    # x(0,0) slab A: padded rows 0..33
B_ROWS = HP - 32               # x(0,0) slab B: padded rows 32..57
WCOLS = 9 * 128                # one (ci_chunk, co_chunk) weight slice

# packed input tensors: name -> (x-columns, carries-w-slice)
IN_SPECS = {
    "in0a": (10 * WP, True),      # x00 rows 0..9 + w(0,0)
    "in0b": (26 * WP, False),     # x00 rows 8..33
    "in5": (IMG, False),          # x00 shifted
    "in1": (IMG, True),           # x10        + w(1,0)
    "in6": (IMG, False),          # x10 shifted
    "in2": (B_ROWS * WP, True),   # x00b       + w(0,1)
    "in3": (IMG, True),           # x01        + w(1,1)
    "in4": (IMG, False),          # x11
    "in7": (IMG, False),          # x01 shifted
    "in8": (IMG, False),          # x11 shifted
}
K_ALIGNED = [0, 2, 3, 5, 6, 8]   # kw in {0, 2}: 4B-aligned in the plain copy
K_SHIFTED = [1, 4, 7]            # kw == 1: read the shifted copy at kw=0

_BF16 = ml_dtypes.bfloat16


def _build_program():
    import concourse.bass as bass
    import concourse.mybir as mybir
    from concourse import bacc
    from concourse.tile import TileContext

    nc = bacc.Bacc("TRN2", target_bir_lowering=False, debug=False)

    in_h = {
        name: nc.dram_tensor(
            name, [128, xc + (WCOLS if has_w else 0)],
            mybir.dt.bfloat16, kind="ExternalInput",
        )
        for name, (xc, has_w) in IN_SPECS.items()
    }
    b_h = nc.dram_tensor("b", [128, 2], mybir.dt.float32, kind="ExternalInput")
    y_h = nc.dram_tensor(
        "y", [IMG_PER_CORE, 2, 128, H, W], mybir.dt.int32, kind="ExternalOutput"
    )

    with TileContext(nc) as tc:
        with (
            tc.tile_pool(name="const", bufs=1) as const_pool,
            tc.tile_pool(name="xin", bufs=1) as x_pool,
            tc.tile_pool(name="psum", bufs=5, space="PSUM") as psum_pool,
            tc.tile_pool(name="warm", bufs=1, space="PSUM") as warm_pool,
            tc.tile_pool(name="outs", bufs=2) as out_pool,
        ):
            # PE warm-up: junk matmuls on a zeroed tile while the input
            # DMAs land, so the HAM clock gate is at 8/8 (2.4 GHz) when
            # the real matmuls start.
            wz = const_pool.tile([128, 128], mybir.dt.bfloat16)
            nc.vector.memset(wz[:, :], 0.0)
            wps = warm_pool.tile([128, 128], mybir.dt.float32)
            for i in range(N_WARM):
                nc.tensor.matmul(
                    wps[:, :], wz[:, :], wz[:, :],
                    start=True, stop=True,
                )

            in_sb = {
                name: x_pool.tile(
                    [128, int(in_h[name].shape[1])], mybir.dt.bfloat16,
                    tag=name, name=f"t_{name}",
                )
                for name in IN_SPECS
            }
            b_sb = const_pool.tile([128, 2], mybir.dt.float32)

            # One input issue stream in first-needed order: DMA queues are
            # FIFO, so earlier transfers drain at full bandwidth before
            # later ones start, instead of fair-sharing with
            # not-yet-needed data.
            nc.scalar.dma_start(b_sb[:, :], b_h.ap())
            for name in IN_SPECS:
                nc.sync.dma_start(in_sb[name][:, :], in_h[name].ap())

            # weight slice views: (ci, co) -> [128, 9*128] region
            w_sb = {
                (0, 0): in_sb["in0a"][:, 10 * WP:],
                (1, 0): in_sb["in1"][:, IMG:],
                (0, 1): in_sb["in2"][:, B_ROWS * WP:],
                (1, 1): in_sb["in3"][:, IMG:],
            }

            def xview(name, cols):
                return in_sb[name][:, :cols].rearrange("p (r c) -> p r c", c=WP)

            x00a_v = xview("in0a", 10 * WP)       # padded rows 0..9
            x00m_v = xview("in0b", 26 * WP)       # padded rows 8..33
            x00b_v = xview("in2", B_ROWS * WP)    # padded rows 32..57
            x_sb = {
                (1, 0): xview("in1", IMG),
                (0, 1): xview("in3", IMG),
                (1, 1): xview("in4", IMG),
            }
            x_shift = {
                (0, 0): xview("in5", IMG),
                (1, 0): xview("in6", IMG),
                (0, 1): xview("in7", IMG),
                (1, 1): xview("in8", IMG),
            }

            def rhs_ap(ci, img, r0, rows, kh, kw):
                r = r0 + kh
                if kw == 1:
                    return x_shift[ci, img][:, r:r + rows, 0:W]
                if (ci, img) == (0, 0):
                    if r + rows <= 10:
                        return x00a_v[:, r:r + rows, kw:kw + W]
                    if r + rows <= A_ROWS:
                        return x00m_v[:, r - 8:r - 8 + rows, kw:kw + W]
                    return x00b_v[:, r - 32:r - 32 + rows, kw:kw + W]
                return x_sb[ci, img][:, r:r + rows, kw:kw + W]

            def mm(ps, ci, co, img, r0, rows, ks, start, stop):
                for i, k in enumerate(ks):
                    kh, kw = divmod(k, 3)
                    nc.tensor.matmul(
                        ps[:, :],
                        w_sb[ci, co][:, k * 128:(k + 1) * 128],
                        rhs_ap(ci, img, r0, rows, kh, kw),
                        start=start and i == 0,
                        stop=stop and i == len(ks) - 1,
                    )

            def epilogue(ps, co, img, r0, rows):
                n = rows * W
                ot = out_pool.tile([128, CHUNK], mybir.dt.int32, tag="ot")
                nc.vector.tensor_scalar_add(
                    ot[:, :n], ps[:, :], b_sb[:, co:co + 1]
                )
                dst = y_h.ap()[img, co].rearrange("p h w -> p (h w)")[
                    :, r0 * W:r0 * W + n
                ]
                nc.sync.dma_start(dst, ot[:, :n])

            # First plane: sweep ci=0 over the first 4 chunks before any
            # ci=1 matmul, aligned taps before shifted taps, so the PE
            # only gates on the first packed transfer (x00a + w00) and the
            # shifted copy (in5) has time to arrive.
            HEAD = 4
            head_ps = []
            for pc in range(HEAD):
                ps = psum_pool.tile([128, CHUNK], mybir.dt.float32, tag="ps",
                                    name=f"ps_h{pc}")
                head_ps.append(ps)
                mm(ps, 0, 0, 0, pc * ROWS_PER_CHUNK, ROWS_PER_CHUNK,
                   K_ALIGNED, start=True, stop=False)
            for pc in range(HEAD):
                mm(head_ps[pc], 0, 0, 0, pc * ROWS_PER_CHUNK, ROWS_PER_CHUNK,
                   K_SHIFTED, start=False, stop=False)
            for pc in range(HEAD):
                mm(head_ps[pc], 1, 0, 0, pc * ROWS_PER_CHUNK, ROWS_PER_CHUNK,
                   K_ALIGNED + K_SHIFTED, start=False, stop=True)
                epilogue(head_ps[pc], 0, 0, pc * ROWS_PER_CHUNK,
                         ROWS_PER_CHUNK)

            # chunk row-splits per (img, co) plane; the globally last chunk
            # is split [6, 2] so the final PSUM->SBUF->HBM drain is short
            for img in range(IMG_PER_CORE):
                for co in range(2):
                    if img == 0 and co == 0:
                        chunks = [(pc * ROWS_PER_CHUNK, ROWS_PER_CHUNK)
                                  for pc in range(HEAD, N_CHUNKS)]
                    elif img == IMG_PER_CORE - 1 and co == 1:
                        chunks = [(pc * ROWS_PER_CHUNK, ROWS_PER_CHUNK)
                                  for pc in range(N_CHUNKS - 1)]
                        chunks += [(48, 6), (54, 2)]
                    else:
                        chunks = [(pc * ROWS_PER_CHUNK, ROWS_PER_CHUNK)
                                  for pc in range(N_CHUNKS)]
                    for r0, rows in chunks:
                        ps = psum_pool.tile([128, CHUNK], mybir.dt.float32,
                                            tag="ps", name=f"ps_{img}_{co}_{r0}")
                        mm(ps[:, :rows * W], 0, co, img, r0, rows,
                           K_ALIGNED + K_SHIFTED, start=True, stop=False)
                        mm(ps[:, :rows * W], 1, co, img, r0, rows,
                           K_ALIGNED + K_SHIFTED, start=False, stop=True)
                        epilogue(ps[:, :rows * W], co, img, r0, rows)

    nc.compile()
    return nc


_NC = None
LAST_RESULT = None  # BassKernelResults of the most recent run (for harnesses)


def kernel(x_int: np.ndarray, weight_int: np.ndarray, bias_int: np.ndarray):
    from concourse.bass_utils import run_bass_kernel_spmd

    global _NC, LAST_RESULT
    if _NC is None:
        _NC = _build_program()
    nc = _NC

    x_int = np.asarray(x_int)
    weight_int = np.asarray(weight_int)
    bias_int = np.asarray(bias_int)

    # x: pad to 58x58, cast to bf16, split channels into two 128-partition
    # chunks: x_flat[b, ci_chunk, 128, 58, 58]
    x_pad = np.zeros((B, C, HP, WP), dtype=_BF16)
    x_pad[:, :, 1:57, 1:57] = x_int.astype(_BF16)
    x_r = x_pad.reshape(B, 2, 128, HP, WP)
    # left-shift-by-one copy: xs[.., c] = x[.., c+1]
    x_s = np.zeros_like(x_r)
    x_s[..., :WP - 1] = x_r[..., 1:]
    x_flat = x_r.reshape(B, 2, 128, IMG)
    x_sflat = x_s.reshape(B, 2, 128, IMG)

    # w[co,ci,kh,kw] -> [ci_part, (ci_chunk, co_chunk, k, co_part)]
    w_t = (
        weight_int.astype(_BF16)
        .reshape(2, 128, 2, 128, 9)          # [co_c, co_p, ci_c, ci_p, k]
        .transpose(3, 2, 0, 4, 1)            # [ci_p, ci_c, co_c, k, co_p]
        .reshape(128, 2 * 2 * 9 * 128)
    )

    def w_slice(ci, co):
        s = (ci * 2 + co) * WCOLS
        return w_t[:, s:s + WCOLS]

    b_t = np.ascontiguousarray(
        bias_int.astype(np.float32).reshape(2, 128).T
    )

    def cat(*arrs):
        return np.ascontiguousarray(np.concatenate(arrs, axis=1))

    in_maps = []
    for c in range(N_CORES):
        xs = x_flat[c * IMG_PER_CORE:(c + 1) * IMG_PER_CORE]
        ss = x_sflat[c * IMG_PER_CORE:(c + 1) * IMG_PER_CORE]
        in_maps.append(
            {
                "in0a": cat(xs[0, 0][:, :10 * WP], w_slice(0, 0)),
                "in0b": np.ascontiguousarray(
                    xs[0, 0][:, 8 * WP:A_ROWS * WP]),
                "in5": np.ascontiguousarray(ss[0, 0]),
                "in1": cat(xs[0, 1], w_slice(1, 0)),
                "in6": np.ascontiguousarray(ss[0, 1]),
                "in2": cat(xs[0, 0][:, 32 * WP:], w_slice(0, 1)),
                "in3": cat(xs[1, 0], w_slice(1, 1)),
                "in4": np.ascontiguousarray(xs[1, 1]),
                "in7": np.ascontiguousarray(ss[1, 0]),
                "in8": np.ascontiguousarray(ss[1, 1]),
                "b": b_t,
            }
        )

    res = run_bass_kernel_spmd(nc, in_maps, core_ids=list(range(N_CORES)))
    LAST_RESULT = res

    y = np.empty((B, C, H, W), dtype=np.int32)
    for c in range(N_CORES):
        yc = res.results[c]["y"]  # [img, co_chunk, 128, H, W]
        for img in range(IMG_PER_CORE):
            y[c * IMG_PER_CORE + img] = yc[img].reshape(C, H, W)
    return y
